# revision 15
# baseline (speedup 1.0000x reference)
"""ConvLRU model kernel for 8 Trainium2 NeuronCores (Bass/Tile).

Sharding: width-parallel. Core k owns output columns [8k, 8k+8) of every frame.
 - conv feature extraction: fp8e4 DoubleRow matmuls (2 MACs/cell/cycle).
   Contraction packs (channel x 2 horizontal tap copies) x (2 vertical taps
   via the DoubleRow i-dim, an overlapping row-shifted read AP); output packs
   (channel x 2 vertical tap groups).  One [128,2,128] weight covers a 4x2
   tap block -> 8 slides x 2 regions of 280-col matmuls per frame (16 MMs vs
   32 for bf16).  Weights are host-scaled by 2^11 to dodge e4m3 subnormals;
   the 2^-11 rides the merge.  Residual x ships separately in bf16.
 - GroupNorm: per-core partial sums fused into the merge DVE (accum_out),
   two AllGathers (halves) + on-device reduce; a dummy collective fired at
   t0 absorbs the ~30-45us comm-init barrier and the ~11us first-collective
   trigger delay so stats resolve right as the conv drains.
 - gated pointwise convs + residual: all-bf16 block-diag paired matmuls;
   residual prefilled into u tiles by DMA from the bf16 x copy (gpsimd queue).
 - LRU over L: y_l = sum_j G[l,j,c] u_j, lower-triangular per-channel G
   precomputed on host -> bf16 matmuls; G ships as one bulk DMA.

kernel(**inputs) takes FULL inputs, returns FULL [B, C, L, H, W] output.
"""
import os
import sys

if os.environ.get("JAX_PLATFORMS") == "cpu":
    # the bass kernel can only execute on the neuron cores
    os.environ["JAX_PLATFORMS"] = "axon"

try:
    import concourse.bass  # noqa: F401
except ImportError:
    sys.path.insert(0, "/opt/trn_rl_repo")

import ml_dtypes
import numpy as np
import concourse.bacc as bacc
import concourse.tile as tile
from concourse import mybir
from concourse.bass_utils import run_bass_kernel_spmd

dt_ = mybir.dt

B, C, L, H, W = 2, 64, 16, 64, 64
KK = 7
PAD = 3
NCORES = 8
WSL = W // NCORES            # 8 output cols per core
WP8 = 16                     # fp8 tile width (8 + 2*3 halo + align pad)
HP = H + 2 * PAD + 1         # 71 rows (region views reach row 70)
NFR = B * L                  # 32 frames
NPP = L // 2                 # 8 l-pairs per batch
NPAIRS = B * NPP             # 16 pair tiles
SPOS = H * WSL               # 512 positions per frame-slice
NROW8 = 34                   # region window rows (g=1 uses j<=33)
RP8 = NROW8 * WSL            # 272 region positions
NGM = B * (NPP * (NPP + 1) // 2)  # 72 LRU matrices
NELEM = 16 * H * W           # groupnorm group element count
NSC = 6                      # per-eighth stats cols: 4 mu parts + 2 m2
WSCALE = 2.0 ** 11           # fp8 weight pre-scale (merge applies 2^-11)

bf16 = ml_dtypes.bfloat16
fp8 = ml_dtypes.float8_e4m3


def _shifted_pair(ap, stride_elems):
    """[p, R, Cc] view -> [p, 2, R, Cc] where the new dim advances by
    stride_elems (overlapping read AP = the DoubleRow dy shift)."""
    ap = ap.unsqueeze(1)
    v = ap.ap
    v[1] = (stride_elems, 2)
    ap.ap = v
    return ap


# ---------------------------------------------------------------- host prep

def _host_prep(inputs):
    x = np.asarray(inputs["x"], np.float32)
    W_sp = np.asarray(inputs["W_sp"], np.float32)
    W_dc = np.asarray(inputs["W_dc"], np.float32)
    W_in = np.asarray(inputs["W_in"], np.float32)
    W_out = np.asarray(inputs["W_out"], np.float32)
    b_dc = np.asarray(inputs["b_dc"], np.float32)
    b_in = np.asarray(inputs["b_in"], np.float32)
    b_out = np.asarray(inputs["b_out"], np.float32)
    gn_g = np.asarray(inputs["gn_g"], np.float32)
    gn_b = np.asarray(inputs["gn_b"], np.float32)

    xf = x.transpose(0, 2, 1, 3, 4).reshape(NFR, C, H, W)
    xp = np.pad(xf, ((0, 0), (0, 0), (0, 0), (PAD, PAD + 3)), mode="wrap")
    xp = np.pad(xp, ((0, 0), (0, 0), (PAD, PAD + 1), (0, 0)), mode="edge")
    # xp: [32, 64, 71, 73]

    xin, xres = [], []
    for k in range(NCORES):
        sl = xp[:, :, :, WSL * k : WSL * k + WP8 + 1]     # [32, 64, 71, 17]
        t = np.empty((NFR, 2 * C, HP, WP8), np.float32)
        t[:, :C] = sl[:, :, :, 0:WP8]                      # dx = 0
        t[:, C:] = sl[:, :, :, 1 : WP8 + 1]                # dx = 1
        xin.append(np.ascontiguousarray(np.clip(t, -240, 240).astype(fp8)))
        xres.append(
            np.ascontiguousarray(
                xf[:, :, :, WSL * k : WSL * k + WSL].astype(bf16)
            )
        )

    W_eff = np.einsum("oc,cixy->oixy", W_dc, W_sp)        # [C_o, C_i, ky, kx]
    # tap block per (sv, sh) slide: ky = sv + 2g + i, kx = sh + dx
    wconv = np.zeros((2 * C, 2, 4, 2, 2 * C), np.float32)
    for svi in range(2):
        for shi in range(4):
            sv, sh = 4 * svi, 2 * shi
            for dx in range(2):
                for g in range(2):
                    for i in range(2):
                        ky = sv + 2 * g + i
                        kx = sh + dx
                        if ky < KK and kx < KK:
                            wconv[dx * C : dx * C + C, svi, shi, i,
                                  g * C : g * C + C] = W_eff[:, :, ky, kx].T * WSCALE
    wconv_d = np.ascontiguousarray(np.clip(wconv, -240, 240).astype(fp8))

    win1 = np.zeros((2 * C, 2 * C), np.float32)
    win2 = np.zeros((2 * C, 2 * C), np.float32)
    win1[:C, :C] = W_in[:C].T
    win1[C:, C:] = W_in[:C].T
    win2[:C, :C] = W_in[C:].T
    win2[C:, C:] = W_in[C:].T
    woutm = np.zeros((2 * C, 2 * C), np.float32)
    woutm[:C, :C] = W_out.T
    woutm[C:, C:] = W_out.T

    gsel = np.zeros((2 * C, 2 * C), np.float32)
    grp = np.arange(C) // 16
    same = grp[:, None] == grp[None, :]
    gsel[:C, :C] = same
    gsel[C:, C:] = same

    vecs = np.zeros((2 * C, 6), np.float32)
    vecs[:, 0] = np.concatenate([b_dc, b_dc])
    vecs[:, 1] = np.concatenate([b_in[:C], b_in[:C]])
    vecs[:, 2] = np.concatenate([b_in[C:], b_in[C:]])
    vecs[:, 3] = np.concatenate([b_out, b_out])
    vecs[:, 4] = np.concatenate([gn_g, gn_g])
    vecs[:, 5] = np.concatenate([gn_b, gn_b])

    return (
        xin,
        xres,
        wconv_d,
        np.ascontiguousarray(win1.astype(bf16)),
        np.ascontiguousarray(win2.astype(bf16)),
        np.ascontiguousarray(woutm.astype(bf16)),
        gsel,
        vecs,
    )


def _host_lru_g(inputs):
    dt = np.asarray(inputs["dt"], np.float64)
    nu = np.exp(np.asarray(inputs["nu_log"], np.float64))
    th = np.exp(np.asarray(inputs["theta_log"], np.float64))
    cr = np.asarray(inputs["c_re"], np.float64)
    ci = np.asarray(inputs["c_im"], np.float64)
    ds = np.asarray(inputs["d_skip"], np.float64)

    decay = np.exp(-nu[None, None, :] * dt[:, :, None])
    phase = th[None, None, :] * dt[:, :, None]
    lam = decay * np.exp(1j * phase)
    gam = np.sqrt(np.maximum(1.0 - decay**2, 1e-6))

    G = np.zeros((B, L, L, C), np.float64)
    for b in range(B):
        for l in range(L):
            P = np.ones(C, np.complex128)
            for j in range(l, -1, -1):
                Pg = P * gam[b, j]
                G[b, l, j] = cr * Pg.real + ci * Pg.imag
                P = P * lam[b, j]
        for l in range(L):
            G[b, l, l] += ds
    gm = np.zeros((NGM, 2 * C, 2 * C), np.float64)
    idx = 0
    for b in range(B):
        for p in range(NPP):
            for q in range(p + 1):
                m = np.zeros((2 * C, 2 * C), np.float64)
                for pl in range(2):
                    for pj in range(2):
                        l, j = 2 * p + pl, 2 * q + pj
                        if j <= l:
                            m[pj * C : pj * C + C, pl * C : pl * C + C] = np.diag(
                                G[b, l, j]
                            )
                gm[idx] = m
                idx += 1
    return np.ascontiguousarray(gm.transpose(1, 0, 2).astype(bf16))  # [128, 72, 128]


# ---------------------------------------------------------------- bass build

_NC_CACHE = {}


def _build_nc():
    if "nc" in _NC_CACHE:
        return _NC_CACHE["nc"]
    f32 = dt_.float32
    bf = dt_.bfloat16
    f8 = dt_.float8e4
    Act = mybir.ActivationFunctionType
    Alu = mybir.AluOpType
    DR = mybir.MatmulPerfMode.DoubleRow

    nc = bacc.Bacc("TRN2", target_bir_lowering=False, debug=False)

    xin_d = nc.dram_tensor("xin", [NFR, 2 * C, HP, WP8], f8, kind="ExternalInput")
    xres_d = nc.dram_tensor("xres", [NFR, C, H, WSL], bf, kind="ExternalInput")
    wconv_d = nc.dram_tensor(
        "wconv", [2 * C, 2, 4, 2, 2 * C], f8, kind="ExternalInput"
    )
    gm_d = nc.dram_tensor("gm", [2 * C, NGM, 2 * C], bf, kind="ExternalInput")
    win1_d = nc.dram_tensor("win1", [2 * C, 2 * C], bf, kind="ExternalInput")
    win2_d = nc.dram_tensor("win2", [2 * C, 2 * C], bf, kind="ExternalInput")
    wout_d = nc.dram_tensor("wout", [2 * C, 2 * C], bf, kind="ExternalInput")
    gsel_d = nc.dram_tensor("gsel", [2 * C, 2 * C], f32, kind="ExternalInput")
    vecs_d = nc.dram_tensor("vecs", [2 * C, 6], f32, kind="ExternalInput")
    yout_d = nc.dram_tensor("yout", [B, L, C, H, WSL], f32, kind="ExternalOutput")

    with tile.TileContext(nc) as tc:
        with (
            tc.tile_pool(name="wpool", bufs=1) as wpool,
            tc.tile_pool(name="xpool", bufs=1) as xpool,
            tc.tile_pool(name="gmpool", bufs=1) as gmpool,
            tc.tile_pool(name="ypool", bufs=1) as ypool,
            tc.tile_pool(name="upool", bufs=1) as upool,
            tc.tile_pool(name="spool", bufs=1) as spool,
            tc.tile_pool(name="tpool", bufs=3) as tpool,
            tc.tile_pool(name="mpool", bufs=4) as mpool,
            tc.tile_pool(name="opool", bufs=4) as opool,
            tc.tile_pool(name="dram", bufs=1, space="DRAM") as dram,
        ):
            # dummy collective at t0: absorbs comm-init barrier + first-CC
            # trigger delay ahead of the real stats AllGathers.
            dum_in = dram.tile([2 * C, 1], f32, tag="dumin", name="dumin")
            dum_out = dram.tile([NCORES * 2 * C, 1], f32, tag="dumout", name="dumout")
            nc.gpsimd.collective_compute(
                "AllGather",
                Alu.bypass,
                replica_groups=[list(range(NCORES))],
                ins=[dum_in.opt()],
                outs=[dum_out.opt()],
            )

            xts = [None] * NFR

            def _load_x(fr, split=False):
                xt = xpool.tile([2 * C, HP, WP8], f8, tag=f"x{fr}", name=f"x{fr}")
                if split:
                    nc.sync.dma_start(out=xt[0:C], in_=xin_d[fr, 0:C])
                    nc.scalar.dma_start(out=xt[C:], in_=xin_d[fr, C:])
                else:
                    nc.sync.dma_start(out=xt[:], in_=xin_d[fr])
                xts[fr] = xt

            for fr in range(2):
                _load_x(fr, split=True)
            wconv_t = wpool.tile([2 * C, 2, 4, 2, 2 * C], f8, tag="wc", name="wc")
            nc.scalar.dma_start(out=wconv_t[:], in_=wconv_d[:])
            win1_t = wpool.tile([2 * C, 2 * C], bf, tag="win1")
            nc.scalar.dma_start(out=win1_t[:], in_=win1_d[:])
            win2_t = wpool.tile([2 * C, 2 * C], bf, tag="win2")
            nc.scalar.dma_start(out=win2_t[:], in_=win2_d[:])
            wout_t = wpool.tile([2 * C, 2 * C], bf, tag="wout")
            nc.scalar.dma_start(out=wout_t[:], in_=wout_d[:])
            gsel_t = wpool.tile([2 * C, 2 * C], f32, tag="gsel")
            nc.scalar.dma_start(out=gsel_t[:], in_=gsel_d[:])
            vecs_t = wpool.tile([2 * C, 6], f32, tag="vecs")
            nc.scalar.dma_start(out=vecs_t[:], in_=vecs_d[:])
            eps_t = wpool.tile([2 * C, 1], f32, tag="eps")
            nc.vector.memset(eps_t[:], 1e-5)
            for fr in range(2, 4):
                _load_x(fr, split=True)
            for fr in range(4, NFR):
                _load_x(fr)
            gm_t = gmpool.tile([2 * C, NGM, 2 * C], bf, tag="gm", name="gm")
            nc.scalar.dma_start(out=gm_t[:, : NGM // 2, :], in_=gm_d[:, : NGM // 2, :])
            nc.scalar.dma_start(out=gm_t[:, NGM // 2 :, :], in_=gm_d[:, NGM // 2 :, :])

            NST = 8  # stats eighths: 2 pairs (= 2 conv groups) each
            stats_t = [
                spool.tile([2 * C, NSC], f32, tag=f"st{h}", name=f"st{h}")
                for h in range(NST)
            ]
            cc_in = [
                dram.tile([2 * C, NSC], f32, tag=f"ccin{h}", name=f"ccin{h}")
                for h in range(NST)
            ]
            cc_out = [
                dram.tile([NCORES * 2 * C, NSC], f32, tag=f"ccout{h}", name=f"ccout{h}")
                for h in range(NST)
            ]
            coeffs = [None] * NST
            yts = [None] * NPAIRS
            uts = [None] * NPAIRS

            def _launch_ag(h):
                nc.gpsimd.dma_start(out=cc_in[h][:], in_=stats_t[h][:])
                nc.gpsimd.collective_compute(
                    "AllGather",
                    Alu.bypass,
                    replica_groups=[list(range(NCORES))],
                    ins=[cc_in[h].opt()],
                    outs=[cc_out[h].opt()],
                )

            def _stats_finish(h, psum_pool, ptag):
                red = spool.tile(
                    [2 * C, NCORES, NSC], f32, tag=f"red{h}", name=f"red{h}"
                )
                nc.gpsimd.dma_start(
                    out=red[:],
                    in_=cc_out[h][:].rearrange("(r p) f -> p r f", p=2 * C),
                )
                sr = spool.tile([2 * C, NSC], f32, tag=f"sr{h}", name=f"sr{h}")
                nc.vector.tensor_reduce(
                    out=sr[:],
                    in_=red[:].rearrange("p r f -> p f r"),
                    axis=mybir.AxisListType.X,
                    op=Alu.add,
                )
                sg = psum_pool.tile([2 * C, NSC], f32, tag=ptag, name=f"sg{h}")
                nc.tensor.matmul(sg[:], gsel_t[:], sr[:], start=True, stop=True)
                sgs = spool.tile([2 * C, NSC], f32, tag=f"sgs{h}", name=f"sgs{h}")
                nc.scalar.activation(
                    out=sgs[:], in_=sg[:], func=Act.Copy, scale=1.0 / NELEM
                )
                # mu partial cols are (pair, region)-interleaved: fold regions
                mu_t = spool.tile([2 * C, 2], f32, tag=f"mu{h}", name=f"mu{h}")
                nc.vector.tensor_add(mu_t[:], sgs[:, 0:4:2], sgs[:, 1:4:2])
                m2_t = sgs[:, 4:6]
                var_t = spool.tile([2 * C, 2], f32, tag=f"var{h}", name=f"var{h}")
                nc.vector.tensor_mul(var_t[:], mu_t[:], mu_t[:])
                nc.vector.tensor_sub(var_t[:], m2_t, var_t[:])
                std_t = spool.tile([2 * C, 2], f32, tag=f"std{h}", name=f"std{h}")
                nc.scalar.activation(
                    out=std_t[:], in_=var_t[:], func=Act.Sqrt, bias=eps_t[:], scale=1.0
                )
                a_t = spool.tile([2 * C, 2], f32, tag=f"aaff{h}", name=f"aaff{h}")
                nc.vector.reciprocal(a_t[:], std_t[:])
                nc.vector.tensor_scalar_mul(a_t[:], a_t[:], vecs_t[:, 4:5])
                b_t = spool.tile([2 * C, 2], f32, tag=f"baff{h}", name=f"baff{h}")
                nc.vector.tensor_mul(b_t[:], mu_t[:], a_t[:])
                nc.vector.tensor_scalar(
                    out=b_t[:],
                    in0=b_t[:],
                    scalar1=-1.0,
                    scalar2=vecs_t[:, 5:6],
                    op0=Alu.mult,
                    op1=Alu.add,
                )
                coeffs[h] = (a_t, b_t)

            def _gn_apply(pr):
                h, pc = divmod(pr, 2)
                a_t, b_t = coeffs[h]
                nc.vector.tensor_scalar(
                    out=yts[pr][:],
                    in0=yts[pr][:],
                    scalar1=a_t[:, pc : pc + 1],
                    scalar2=b_t[:, pc : pc + 1],
                    op0=Alu.mult,
                    op1=Alu.add,
                )

            hts = {}

            def _chain_pe(pr, wpsum):
                yt = yts[pr]
                h1 = wpsum.tile([2 * C, SPOS], f32, tag="h", name=f"h1_{pr}")
                nc.tensor.matmul(h1[:], win1_t[:], yt[:], start=True, stop=True)
                h2 = wpsum.tile([2 * C, SPOS], f32, tag="h", name=f"h2_{pr}")
                nc.tensor.matmul(h2[:], win2_t[:], yt[:], start=True, stop=True)
                hts[pr] = (h1, h2)

            def _chain_rest(pr, opsum):
                h1, h2 = hts[pr]
                sig = tpool.tile([2 * C, SPOS], f32, tag="sig", name=f"sig{pr}")
                nc.scalar.activation(
                    out=sig[:],
                    in_=h2[:],
                    func=Act.Sigmoid,
                    bias=vecs_t[:, 2:3],
                    scale=1.0,
                )
                zt = tpool.tile([2 * C, SPOS], bf, tag="z", name=f"z{pr}")
                nc.vector.scalar_tensor_tensor(
                    out=zt[:],
                    in0=h1[:],
                    scalar=vecs_t[:, 1:2],
                    in1=sig[:],
                    op0=Alu.add,
                    op1=Alu.mult,
                )
                z2 = opsum.tile([2 * C, SPOS], f32, tag="z2", name=f"z2_{pr}")
                nc.tensor.matmul(z2[:], wout_t[:], zt[:], start=True, stop=True)
                # residual was DMA-prefilled into ut during the conv phase
                ut = uts[pr]
                nc.vector.scalar_tensor_tensor(
                    out=ut[:],
                    in0=z2[:],
                    scalar=vecs_t[:, 3:4],
                    in1=ut[:],
                    op0=Alu.add,
                    op1=Alu.add,
                )

            def _chain_batch(p0, wpsum, opsum, filler=None):
                _chain_pe(p0, wpsum)
                _chain_pe(p0 + 1, wpsum)
                # PE-queue filler between the h-matmuls and the z2s: the
                # queue is in-order, so the sigmoid/gate round-trip would
                # otherwise idle the PE
                if filler is not None:
                    filler()
                _chain_rest(p0, opsum)
                _chain_rest(p0 + 1, opsum)

            def _lru_row(b, p, lpsum):
                gidx_base = b * (NPP * (NPP + 1) // 2)
                lp = lpsum.tile([2 * C, SPOS], f32, tag="lp", name=f"lp{b}_{p}")
                for q in range(p + 1):
                    gidx = gidx_base + p * (p + 1) // 2 + q
                    nc.tensor.matmul(
                        lp[:],
                        gm_t[:, gidx, :],
                        uts[b * NPP + q][:],
                        start=(q == 0),
                        stop=(q == p),
                    )
                yo = opool.tile([2 * C, SPOS], f32, tag="yo", name=f"yo{b}_{p}")
                nc.scalar.activation(out=yo[:], in_=lp[:], func=Act.Copy)
                nc.sync.dma_start(out=yout_d[b, 2 * p : 2 * p + 2], in_=yo[:])

            # ---- conv phase: slide-outer over 2-frame blocks; 2 region PSUM
            # tiles per frame; fp8 DoubleRow 4x2-tap-packed matmuls.
            def _conv_frame_group(frames, cpsum):
                pss = {}
                for fr in frames:
                    pss[fr] = [
                        cpsum.tile([2 * C, RP8], f32, tag="cps", name=f"cps{fr}_{r}")
                        for r in range(2)
                    ]
                for t in range(8):
                    svi, shi = t // 4, t % 4
                    sv, sh = 4 * svi, 2 * shi
                    wap = wconv_t[:, svi, shi]
                    for fr in frames:
                        for r in range(2):
                            mov = _shifted_pair(
                                xts[fr][
                                    :, 32 * r + sv : 32 * r + sv + NROW8, sh : sh + WSL
                                ],
                                WP8,
                            )
                            nc.tensor.matmul(
                                pss[fr][r][:],
                                wap,
                                mov,
                                start=(t == 0),
                                stop=(t == 7),
                                perf_mode=DR,
                            )
                for fr in frames:
                    pr, hh = divmod(fr, 2)
                    h, pc = divmod(pr, 2)
                    if hh == 0:
                        yt = ypool.tile([2 * C, SPOS], bf, tag=f"y{pr}", name=f"y{pr}")
                        yts[pr] = yt
                        ut = upool.tile([2 * C, SPOS], bf, tag=f"u{pr}", name=f"u{pr}")
                        uts[pr] = ut
                        # residual prefill by DMA from the bf16 x copy
                        # (gpsimd queue: idle outside the cc chain)
                        nc.gpsimd.dma_start(out=ut[0:C, :], in_=xres_d[fr])
                        nc.gpsimd.dma_start(out=ut[C:, :], in_=xres_d[fr + 1])
                    yt = yts[pr]
                    po = C * hh        # partition base of this frame's output
                    for r in range(2):
                        # main block (g=0, ky offsets {0,1}) at partitions
                        # 0:64 cols [0:256]; far block (g=1, ky offsets
                        # {2,3}) at partitions 64:128 cols [16:272].  The
                        # ACT evac folds the 2^-11 weight descale + conv
                        # bias and shifts partitions for the frame parity.
                        tmp = mpool.tile([2 * C, 256], f32, tag="mrg")
                        if hh == 0:
                            nc.scalar.activation(
                                out=tmp[0:C, :],
                                in_=pss[fr][r][C:, 16:272],
                                func=Act.Identity,
                                bias=vecs_t[C:, 0:1],
                                scale=1.0 / WSCALE,
                            )
                            dve_in = pss[fr][r][0:C, 0:256]
                        else:
                            nc.scalar.activation(
                                out=tmp[C:, :],
                                in_=pss[fr][r][0:C, 0:256],
                                func=Act.Identity,
                                bias=vecs_t[0:C, 0:1],
                                scale=1.0 / WSCALE,
                            )
                            dve_in = pss[fr][r][C:, 16:272]
                        nc.vector.scalar_tensor_tensor(
                            out=yt[po : po + C, 256 * r : 256 * r + 256],
                            in0=dve_in,
                            scalar=1.0 / WSCALE,
                            in1=tmp[po : po + C, :],
                            op0=Alu.mult,
                            op1=Alu.add,
                            accum_out=stats_t[h][
                                po : po + C, 2 * pc + r : 2 * pc + r + 1
                            ],
                        )
                    if hh == 1:
                        # square pass for m2 stats over the whole pair tile
                        trash = tpool.tile([2 * C, SPOS], bf, tag="trash")
                        nc.vector.scalar_tensor_tensor(
                            out=trash[:],
                            in0=yt[:],
                            scalar=1.0,
                            in1=yt[:],
                            op0=Alu.bypass,
                            op1=Alu.mult,
                            accum_out=stats_t[h][:, 4 + pc : 4 + pc + 1],
                        )

            with (
                tc.tile_pool(name="cpsum", bufs=8, space="PSUM") as cpsum,
            ):
                for g0 in range(0, NFR, 2):
                    _conv_frame_group(range(g0, g0 + 2), cpsum)
                    if (g0 + 2) % 4 == 0:
                        _launch_ag((g0 + 2) // 4 - 1)

            with (
                tc.tile_pool(name="wpsum", bufs=4, space="PSUM") as wpsum,
                tc.tile_pool(name="opsum", bufs=2, space="PSUM") as opsum,
                tc.tile_pool(name="lpsum", bufs=2, space="PSUM") as lpsum,
            ):
                # GN applies woven right before the chains that consume them;
                # LRU rows slot in as PE filler for the gate round-trips.
                def _rows(rows):
                    return lambda: [_lru_row(b, p, lpsum) for b, p in rows]

                fills = {
                    0: [],
                    2: [(0, 0)],
                    4: [(0, 1)],
                    6: [(0, 2), (0, 3)],
                    8: [(0, 4)],
                    10: [(0, 5), (0, 6)],
                    12: [(0, 7), (1, 0), (1, 1)],
                    14: [(1, 2), (1, 3)],
                }
                for p0 in range(0, 16, 2):
                    _stats_finish(p0 // 2, wpsum, "h")
                    _gn_apply(p0)
                    _gn_apply(p0 + 1)
                    _chain_batch(
                        p0, wpsum, opsum, _rows(fills[p0]) if fills[p0] else None
                    )
                for p in range(4, 8):
                    _lru_row(1, p, lpsum)

    nc.finalize()
    _NC_CACHE["nc"] = nc
    return nc


# ---------------------------------------------------------------- entry point

def kernel(**inputs):
    xin, xres, wconv_d, win1, win2, woutm, gsel, vecs = _host_prep(inputs)
    gm = _host_lru_g(inputs)
    nc = _build_nc()

    shared = {
        "wconv": wconv_d,
        "gm": gm,
        "win1": win1,
        "win2": win2,
        "wout": woutm,
        "gsel": gsel,
        "vecs": vecs,
    }
    in_maps = [dict(shared, xin=xin[k], xres=xres[k]) for k in range(NCORES)]
    res = run_bass_kernel_spmd(nc, in_maps, list(range(NCORES)))

    full = np.zeros((B, C, L, H, W), np.float32)
    for k in range(NCORES):
        yo = res.results[k]["yout"]  # [B, L, C, H, WSL]
        full[:, :, :, :, WSL * k : WSL * k + WSL] = yo.transpose(0, 2, 1, 3, 4)
    return full
